# revision 30
# baseline (speedup 1.0000x reference)
"""Trainium2 Bass kernel for nn_EquivariantUpdate (GNN message passing).

Strategy (edge-parallel across 8 NeuronCores, SPMD single program):
  - Host splits nodes into 8 contiguous ranges balanced by edge count; core c
    owns its node range and all edges whose row falls in it, so the
    segment-sum is core-local (no collectives).
  - Host precomputes the node-level tables A = h@W1[:128] and
    B = h@W1[128:256] (as in the prior baseline) and marshals all per-edge
    data into a windowed layout: nodes pack into variable-span windows
    (<=127 nodes, TPW=17 tiles of 128 edge slots each); every window gets
    exactly TPW tiles, zero-padded.  Per-edge tensors are laid out
    [feature/slot, edge] so the device consumes them directly:
      * bedT  fp8 [128, T*128]  = B[col_e] per edge slot (feature-major)
      * rtT   fp8 [128, T*128]  = one-hot(local row) with row 127 = attr_e
      * A_sb  fp8 [128, NW*128] = window A slices, slot 127 = W1 attr row
      * cdt   bf16 [128, T, 3]  = masked coord_diff, lrow bf16 [128, T]
  - Device per 512-edge chunk: p1(psum) = A_sb-window^T @ rtT (adds
    A[row]+attr*w1a) + I @ bedT (adds B[col]); both silu layers run on the
    ACT engine at 1536-wide calls in a one-group-lag software pipeline so
    ACT (the wall) never idles; W2 via fp8 matmul; scale = x2-tile^T @ w3
    (fp8, 4 small MMs/chunk); cds = cdt * psc on DVE; segment-sum via
    per-tile one-hot matmuls (otr built on DVE from lrow+iota) accumulated
    in a [128,3] psum per window.
  - w3 is prescaled by 2**16/100 on host (fp8 range); the final output
    stage computes out = agg * (mask/2**16) + coord*mask and the host
    reassembles the variable window spans.
"""

import sys
import os

sys.path.insert(0, "/opt/trn_rl_repo")

import numpy as np
import ml_dtypes

BF16 = ml_dtypes.bfloat16
FP8 = ml_dtypes.float8_e4m3fn

H = 128
NCORES = 8
TILE_E = 128
CHUNK_T = 4          # tiles per MLP chunk (512 edges)
GROUP_C = 3          # chunks per ACT group (1536 edges, 3 psum banks)
GROUP_T = CHUNK_T * GROUP_C          # 16 tiles per group == one window
REGION_T = 48        # tiles per DMA region (3 groups)
TPW = 16             # tiles per window (cap 2048 edges, chunk-aligned)
NORM = 100.0
W3SCALE = 65536.0 / NORM


# ----------------------------------------------------------------------------
# Host-side preparation
# ----------------------------------------------------------------------------

def prep_host(h, coord, edge_index, coord_diff, edge_attr, node_mask,
              edge_mask, W1, b1, W2, b2, W3, ncores=NCORES):
    N = h.shape[0]
    E = edge_index.shape[1]
    row = np.asarray(edge_index[0], dtype=np.int64)
    col = np.asarray(edge_index[1], dtype=np.int64)
    cd = (np.asarray(coord_diff, np.float32)
          * np.asarray(edge_mask, np.float32))          # fold edge_mask

    counts = np.bincount(row, minlength=N)
    cum = np.cumsum(counts)
    bounds = [0]
    for c in range(1, ncores):
        bounds.append(int(np.searchsorted(cum, c * E / ncores)))
    bounds.append(N)

    order = np.argsort(row, kind="stable")
    row_s_all = row[order]

    CAP = TPW * TILE_E
    cum0 = np.concatenate([[0], cum])

    core_windows = []
    for c in range(ncores):
        nlo, nhi = bounds[c], bounds[c + 1]
        wins = []
        pos = nlo
        while pos < nhi:
            k = int(np.searchsorted(cum0, cum0[pos] + CAP,
                                    side="right")) - 1 - pos
            span = min(127, nhi - pos, k)
            assert span >= 1, f"node {pos} exceeds window cap"
            wins.append((pos, span))
            pos += span
        core_windows.append(wins)

    NW = max(len(w) for w in core_windows)
    T = -(-NW * TPW // GROUP_T) * GROUP_T

    regions = []
    t = 0
    while t < T:
        n = min(24 if t < 48 else REGION_T, T - t)
        regions.append((t, n))
        t += n

    h_f = np.asarray(h, np.float32)
    W1 = np.asarray(W1, np.float32)
    w1a_f = W1[2 * H]
    A_tab = h_f @ W1[:H]
    B_tab8 = (h_f @ W1[H:2 * H]).astype(FP8)

    shared = dict(
        b1=np.asarray(b1, np.float32).reshape(H, 1).copy(),
    )
    sc_iota = np.tile(np.arange(TILE_E, dtype=np.float32).astype(BF16),
                      (128, 1))
    sc_ident = np.eye(128, dtype=np.float32).astype(FP8)
    sc_w2 = np.asarray(W2, np.float32).astype(BF16)
    sc_w3 = (np.asarray(W3, np.float32) * W3SCALE).astype(BF16)
    sc_b2 = np.asarray(b2, np.float32).reshape(H, 1)

    attr_f = np.asarray(edge_attr, np.float32).reshape(-1)
    coord_f = np.asarray(coord, np.float32)
    nmask_f = np.asarray(node_mask, np.float32).reshape(-1)

    in_maps = []
    metas = []
    NS = T * TILE_E
    for c in range(ncores):
        wins = core_windows[c]
        col_s = np.zeros(NS, np.int64)
        real = np.zeros(NS, bool)
        lrow_f = np.full(NS, -1.0, np.float32)
        attr_s = np.zeros(NS, np.float32)
        cdt_s = np.zeros((NS, 3), np.float32)
        A_sb = np.zeros((128, NW * H), np.float32)
        coordm = np.zeros((128, NW, 3), np.float32)
        masks = np.zeros((128, NW, 3), np.float32)

        for w, (base, span) in enumerate(wins):
            s = int(np.searchsorted(row_s_all, base, side="left"))
            e = int(np.searchsorted(row_s_all, base + span, side="left"))
            eids = order[s:e]
            m = len(eids)
            assert m <= CAP
            sl = slice(w * CAP, w * CAP + m)
            col_s[sl] = col[eids]
            real[sl] = True
            lrow_f[sl] = (row[eids] - base).astype(np.float32)
            attr_s[sl] = attr_f[eids]
            cdt_s[sl] = cd[eids]

            A_sb[:span, w * H:(w + 1) * H] = A_tab[base:base + span]
            A_sb[127, w * H:(w + 1) * H] = w1a_f
            coordm[:span, w, :] = (coord_f[base:base + span]
                                   * nmask_f[base:base + span, None])
            masks[:span, w, :] = nmask_f[base:base + span, None] / 65536.0

        bedT = B_tab8[col_s].T.copy()            # [128, NS] fp8
        bedT[:, ~real] = FP8(0.0)
        rtT = np.zeros((128, NS), FP8)
        idx = np.nonzero(real)[0]
        rtT[lrow_f[idx].astype(np.int64), idx] = FP8(1.0)
        rtT[127, :] = attr_s.astype(FP8)

        lrow_b = lrow_f.reshape(T, TILE_E).T.astype(BF16)        # [128, T]
        blob = np.zeros((128, 648 + 2 * T), np.uint8)
        blob[:, 0:256] = sc_iota.view(np.uint8).reshape(128, 256)
        blob[:, 256:384] = sc_ident.view(np.uint8).reshape(128, 128)
        blob[:, 384:640] = sc_w2.view(np.uint8).reshape(128, 256)
        blob[:, 640:642] = sc_w3.view(np.uint8).reshape(128, 2)
        blob[:, 644:648] = sc_b2.view(np.uint8).reshape(128, 4)
        blob[:, 648:] = np.ascontiguousarray(lrow_b).view(np.uint8)
        im = dict(
            bedT=np.ascontiguousarray(bedT),
            rtT=np.ascontiguousarray(rtT),
            A_sb=np.ascontiguousarray(A_sb.astype(FP8)),
            cblob=np.ascontiguousarray(blob),
            cdt=np.ascontiguousarray(
                cdt_s.reshape(T, TILE_E, 3).transpose(1, 0, 2)
                .reshape(128, T * 3).astype(BF16)),
            coordm=np.ascontiguousarray(coordm.reshape(128, NW * 3)),
            masks=np.ascontiguousarray(masks.reshape(128, NW * 3)),
        )
        im.update(shared)
        in_maps.append(im)
        metas.append(dict(wins=wins))

    dims = dict(T=T, NW=NW, regions=regions, N=N)
    return in_maps, metas, dims


# ----------------------------------------------------------------------------
# Bass program
# ----------------------------------------------------------------------------

def build_program(dims):
    from concourse import bass, bacc, tile, mybir

    T, NW = dims["T"], dims["NW"]
    regions = dims["regions"]
    CH_E = CHUNK_T * TILE_E                     # 512
    GR_E = GROUP_T * TILE_E                     # 1536
    RE_MAX = REGION_T * TILE_E                  # 6144
    f32 = mybir.dt.float32
    bf16 = mybir.dt.bfloat16
    fp8 = mybir.dt.float8e4
    n_real_t = NW * TPW

    nc = bacc.Bacc("TRN2", target_bir_lowering=False, debug=False,
                   num_swdge_queues=1, dynamic_dma_scratch_size=16384)

    def din(name, shape, dt):
        return nc.dram_tensor(name, shape, dt, kind="ExternalInput")

    d_bedT = din("bedT", [128, T * TILE_E], fp8)
    d_rtT = din("rtT", [128, T * TILE_E], fp8)
    d_Asb = din("A_sb", [128, NW * H], fp8)
    d_cblob = din("cblob", [128, 648 + 2 * T], mybir.dt.uint8)
    d_cdt = din("cdt", [128, T, 3], bf16)
    d_coordm = din("coordm", [128, NW * 3], f32)
    d_masks = din("masks", [128, NW * 3], f32)
    d_b1 = din("b1", [H, 1], f32)
    d_out = nc.dram_tensor("out", [128, NW * 3], f32, kind="ExternalOutput")

    SILU = mybir.ActivationFunctionType.Silu
    ABL = set((os.environ.get("KABL") or "").split(","))
    if "noact" in ABL:
        SILU = mybir.ActivationFunctionType.Relu
    AOP = mybir.AluOpType

    def tile_window(t):
        w = t // TPW
        first = (t % TPW == 0)
        last = (t % TPW == TPW - 1) or (t == n_real_t - 1)
        return w, first, last

    with tile.TileContext(nc) as tc:
        with (
            tc.tile_pool(name="const", bufs=1) as cpool,
            tc.tile_pool(name="bed", bufs=3) as bpool,
            tc.tile_pool(name="rtt", bufs=3) as rpool,
            tc.tile_pool(name="otrp", bufs=3) as opool,
            tc.tile_pool(name="x1p", bufs=2) as x1pool,
            tc.tile_pool(name="x2p", bufs=2) as x2pool,
            tc.tile_pool(name="cdsp", bufs=3) as spool,
            tc.tile_pool(name="ps1", bufs=1, space="PSUM") as ps1,
            tc.tile_pool(name="ps2", bufs=1, space="PSUM") as ps2,
            tc.tile_pool(name="psc", bufs=1, space="PSUM") as pscp,
            tc.tile_pool(name="pseg", bufs=1, space="PSUM") as psegp,
        ):
            def load(dram, shape, dt, eng=None):
                t = cpool.tile(shape, dt, tag=f"c_{dram.name}")
                (eng or nc.sync).dma_start(t[:], dram[:])
                return t

            b1 = load(d_b1, [H, 1], f32, eng=nc.scalar)

            # region double-buffered streams
            NREG = len(regions)
            bed_t = {}
            rtt_t = {}

            def load_region(r, split=False):
                t0, nt = regions[r]
                bt = bpool.tile([128, RE_MAX], fp8, tag="bed")
                rt = rpool.tile([128, RE_MAX], fp8, tag="rtt")
                eng2 = nc.scalar if split else nc.sync
                # first region streams in group-sized slices so the first
                # matmuls start as soon as 12 tiles have landed
                step = GROUP_T if r == 0 else nt
                for o in range(0, nt, step):
                    n = min(step, nt - o)
                    esl = slice((t0 + o) * TILE_E, (t0 + o + n) * TILE_E)
                    lsl = slice(o * TILE_E, (o + n) * TILE_E)
                    nc.sync.dma_start(bt[:, lsl], d_bedT[:, esl])
                    eng2.dma_start(rt[:, lsl], d_rtT[:, esl])
                bed_t[r] = bt
                rtt_t[r] = rt

            load_region(0, split=True)
            A_sb = cpool.tile([128, NW * H], fp8, tag="c_A_sb")
            asb_cut = min(8, NW) * H
            nc.sync.dma_start(A_sb[:, 0:asb_cut], d_Asb[:, 0:asb_cut])
            cblob = load(d_cblob, [128, 648 + 2 * T], mybir.dt.uint8)
            iota = cblob[:, 0:256].bitcast(bf16)
            ident = cblob[:, 256:384].bitcast(fp8)
            w2 = cblob[:, 384:640].bitcast(bf16)
            w3 = cblob[:, 640:642].bitcast(bf16)
            b2 = cblob[:, 644:648].bitcast(f32)
            lrow = cblob[:, 648:648 + 2 * T].bitcast(bf16)
            cdt = load(d_cdt, [128, T, 3], bf16)
            if NREG > 1:
                load_region(1, split=True)
            if asb_cut < NW * H:
                nc.sync.dma_start(A_sb[:, asb_cut:], d_Asb[:, asb_cut:])
            # trigger the Silu ACT table load now that all ramp DMA-gen
            # instructions are queued (table DMAs overlap data in flight)
            actwarm = cpool.tile([128, 1], bf16, tag="actwarm")
            nc.vector.memset(actwarm[:], 0.0)
            nc.scalar.activation(actwarm[:], actwarm[:], SILU)
            # keep the PE busy through the DMA ramp so HAM is warm (2.4GHz)
            # when the first real matmuls issue
            pewarm = cpool.tile([128, 128], bf16, tag="pewarm")
            nc.vector.memset(pewarm[:], 0.0)
            pswarm = ps1.tile([128, 512], f32, tag="p1")
            for _ in range(75):
                nc.tensor.matmul(pswarm[:, 0:128], pewarm[:], pewarm[:],
                                 start=True, stop=True,
                                 skip_group_check=True)

            agg = cpool.tile([128, NW * 3], f32, tag="agg")
            nc.vector.memset(agg[:], 0.0)

            pseg_live = [None]
            pending = None

            def emit_tail(g, x1, otr_g):
                t0 = g * GROUP_T
                p2 = ps2.tile([128, GR_E], f32, tag="p2")
                for c in range(GROUP_C):
                    nc.tensor.matmul(p2[:, c * CH_E:(c + 1) * CH_E], w2[:],
                                     x1[:, c * CH_E:(c + 1) * CH_E],
                                     start=True, stop=True,
                                     skip_group_check=True)
                x2 = x2pool.tile([128, GR_E], bf16, tag="x2")
                nc.scalar.activation(x2[:], p2[:], SILU, bias=b2[:])

                psc = pscp.tile([128, GROUP_T], f32, tag="psc")
                for j in range(GROUP_T):
                    if t0 + j >= n_real_t:
                        break
                    nc.tensor.matmul(
                        psc[:, j:j + 1],
                        x2[:, j * TILE_E:(j + 1) * TILE_E],
                        w3[:], start=True, stop=True,
                        skip_group_check=True)
                if "noseg" in ABL:
                    return
                nj = min(GROUP_T, n_real_t - t0)
                cds = spool.tile([128, GROUP_T, 3], fp8, tag="cds")
                nc.vector.tensor_tensor(
                    cds[:, :nj, :], cdt[:, t0:t0 + nj, :],
                    psc[:, :nj, None].broadcast_to([128, nj, 3]),
                    AOP.mult)
                for j in range(nj):
                    gt = t0 + j
                    w, first, last = tile_window(gt)
                    if first:
                        npseg = psegp.tile([128, 3], f32, tag="pseg")
                        pseg_live[0] = npseg
                    ps = pseg_live[0]
                    nc.tensor.matmul(
                        ps[:], otr_g[:, j, :], cds[:, j, :],
                        start=first, stop=last, skip_group_check=True)
                    if last:
                        sl = agg[:, w * 3:w * 3 + 3]
                        nc.vector.tensor_add(sl, sl, ps[:])

            NGRP = T // GROUP_T
            reg_of = {}
            for ri, (rt, rn) in enumerate(regions):
                for tt in range(rt, rt + rn):
                    reg_of[tt] = ri
            for g in range(NGRP):
                t0 = g * GROUP_T
                r = reg_of[t0]
                rt0 = regions[r][0]
                if t0 >= n_real_t:
                    break
                if t0 == rt0 and r + 2 <= NREG - 1:
                    load_region(r + 2)
                off = (t0 - rt0) * TILE_E

                # one-hot [e, slot] for this group's seg matmuls (DVE)
                otr_g = opool.tile([128, GROUP_T, TILE_E], fp8, tag="otr")
                nc.vector.tensor_tensor(
                    otr_g[:],
                    iota[:, None, :].broadcast_to([128, GROUP_T, TILE_E]),
                    lrow[:, t0:t0 + GROUP_T, None].broadcast_to(
                        [128, GROUP_T, TILE_E]),
                    AOP.is_equal)

                p1 = ps1.tile([128, GR_E], f32, tag="p1")
                bed = bed_t[r]
                rtt = rtt_t[r]
                # B-insert opens each bank; A-select accumulates and closes
                for c in range(GROUP_C):
                    co = c * CH_E
                    nc.tensor.matmul(
                        p1[:, co:co + CH_E], ident[:],
                        bed[:, off + co:off + co + CH_E],
                        start=True, stop=False, skip_group_check=True)
                for c in range(GROUP_C):
                    tc0 = t0 + c * CHUNK_T
                    runs = []
                    for t in range(CHUNK_T):
                        gt = tc0 + t
                        w = 0 if gt >= n_real_t else gt // TPW
                        if runs and runs[-1][0] == w:
                            runs[-1][2] = t + 1
                        else:
                            runs.append([w, t, t + 1])
                    co = c * CH_E
                    for w, ta, tb in runs:
                        nc.tensor.matmul(
                            p1[:, co + ta * TILE_E:co + tb * TILE_E],
                            A_sb[:, w * H:(w + 1) * H],
                            rtt[:, off + co + ta * TILE_E:
                                   off + co + tb * TILE_E],
                            start=False, stop=True, skip_group_check=True)

                x1 = x1pool.tile([128, GR_E], bf16, tag="x1")
                nc.scalar.activation(x1[:], p1[:], SILU, bias=b1[:])

                if pending is not None:
                    emit_tail(*pending)
                pending = (g, x1, otr_g)

            coordm = load(d_coordm, [128, NW * 3], f32)
            masks = load(d_masks, [128, NW * 3], f32)
            if pending is not None:
                emit_tail(*pending)

            outs = cpool.tile([128, NW * 3], f32, tag="outs")
            nc.vector.tensor_mul(outs[:], agg[:], masks[:])
            nc.vector.tensor_add(outs[:], outs[:], coordm[:])
            nc.sync.dma_start(d_out[:], outs[:])

    nc.compile()
    return nc


# ----------------------------------------------------------------------------
# Entry point
# ----------------------------------------------------------------------------

LAST_RESULTS = None


def _ensure_ntff_hook():
    """Register the axon NTFF profile hook if the image lacks antenv.axon_hooks."""
    import types
    try:
        from antenv.axon_hooks import get_axon_ntff_profile_hook  # noqa: F401
        return
    except ImportError:
        pass
    holder = {}
    mod = types.ModuleType("antenv.axon_hooks")
    mod.set_axon_ntff_profile_hook = lambda h: holder.__setitem__("h", h)
    mod.get_axon_ntff_profile_hook = lambda: holder.get("h")
    sys.modules["antenv.axon_hooks"] = mod
    try:
        sys.path.insert(0, "/root/.axon_site")
        from trn_agent_boot.trn_boot import _ntff_profile_via_ctypes
        hook = _ntff_profile_via_ctypes("/opt/axon/libaxon_pjrt.so")
        if hook is not None:
            mod.set_axon_ntff_profile_hook(hook)
    except Exception as e:  # degrade to no trace
        print("ntff hook setup failed:", e)
    from concourse import bass_utils as _bu
    _bu.upload_artifacts = lambda tmpdir: f"local:{tmpdir}"


def kernel(**inputs):
    global LAST_RESULTS
    from concourse.bass_utils import run_bass_kernel_spmd

    in_maps, metas, dims = prep_host(**inputs)
    nc = build_program(dims)
    trace = bool(os.environ.get("KERNEL_TRACE"))
    if trace:
        _ensure_ntff_hook()
    tmpdir = os.environ.get("KERNEL_TRACE_DIR") or None
    res = run_bass_kernel_spmd(nc, in_maps, list(range(NCORES)), trace=trace,
                               tmpdir=tmpdir)
    LAST_RESULTS = res

    N = dims["N"]
    NW = dims["NW"]
    out = np.zeros((N, 3), np.float32)
    for c in range(NCORES):
        o = res.results[c]["out"].reshape(128, NW, 3)
        for w, (base, span) in enumerate(metas[c]["wins"]):
            out[base:base + span] = o[:span, w, :]
    return out


# revision 34
# speedup vs baseline: 1.0019x; 1.0019x over previous
"""Trainium2 Bass kernel for nn_EquivariantUpdate (GNN message passing).

Strategy (edge-parallel across 8 NeuronCores, SPMD single program):
  - Host splits nodes into 8 contiguous ranges balanced by edge count; core c
    owns its node range and all edges whose row falls in it, so the
    segment-sum is core-local (no collectives).
  - Host precomputes the node-level tables A = h@W1[:128] and
    B = h@W1[128:256] (as in the prior baseline) and marshals all per-edge
    data into a windowed layout: nodes pack into variable-span windows
    (<=127 nodes, TPW=17 tiles of 128 edge slots each); every window gets
    exactly TPW tiles, zero-padded.  Per-edge tensors are laid out
    [feature/slot, edge] so the device consumes them directly:
      * bedT  fp8 [128, T*128]  = B[col_e] per edge slot (feature-major)
      * rtT   fp8 [128, T*128]  = one-hot(local row) with row 127 = attr_e
      * A_sb  fp8 [128, NW*128] = window A slices, slot 127 = W1 attr row
      * cdt   bf16 [128, T, 3]  = masked coord_diff, lrow bf16 [128, T]
  - Device per 512-edge chunk: p1(psum) = A_sb-window^T @ rtT (adds
    A[row]+attr*w1a) + I @ bedT (adds B[col]); both silu layers run on the
    ACT engine at 1536-wide calls in a one-group-lag software pipeline so
    ACT (the wall) never idles; W2 via fp8 matmul; scale = x2-tile^T @ w3
    (fp8, 4 small MMs/chunk); cds = cdt * psc on DVE; segment-sum via
    per-tile one-hot matmuls (otr built on DVE from lrow+iota) accumulated
    in a [128,3] psum per window.
  - w3 is prescaled by 2**16/100 on host (fp8 range); the final output
    stage computes out = agg * (mask/2**16) + coord*mask and the host
    reassembles the variable window spans.
"""

import sys
import os

sys.path.insert(0, "/opt/trn_rl_repo")

import numpy as np
import ml_dtypes

BF16 = ml_dtypes.bfloat16
FP8 = ml_dtypes.float8_e4m3fn

H = 128
NCORES = 8
TILE_E = 128
CHUNK_T = 4          # tiles per MLP chunk (512 edges)
GROUP_C = 3          # chunks per ACT group (1536 edges, 3 psum banks)
GROUP_T = CHUNK_T * GROUP_C          # 16 tiles per group == one window
REGION_T = 48        # tiles per DMA region (3 groups)
TPW = 12             # tiles per window (cap 1536 edges, == one group)
NORM = 100.0
W3SCALE = 65536.0 / NORM


# ----------------------------------------------------------------------------
# Host-side preparation
# ----------------------------------------------------------------------------

def prep_host(h, coord, edge_index, coord_diff, edge_attr, node_mask,
              edge_mask, W1, b1, W2, b2, W3, ncores=NCORES):
    N = h.shape[0]
    E = edge_index.shape[1]
    row = np.asarray(edge_index[0], dtype=np.int64)
    col = np.asarray(edge_index[1], dtype=np.int64)
    cd = (np.asarray(coord_diff, np.float32)
          * np.asarray(edge_mask, np.float32))          # fold edge_mask

    counts = np.bincount(row, minlength=N)
    cum = np.cumsum(counts)
    bounds = [0]
    for c in range(1, ncores):
        bounds.append(int(np.searchsorted(cum, c * E / ncores)))
    bounds.append(N)

    order = np.argsort(row, kind="stable")
    row_s_all = row[order]

    CAP = TPW * TILE_E
    cum0 = np.concatenate([[0], cum])

    core_windows = []
    for c in range(ncores):
        nlo, nhi = bounds[c], bounds[c + 1]
        wins = []
        pos = nlo
        while pos < nhi:
            k = int(np.searchsorted(cum0, cum0[pos] + CAP,
                                    side="right")) - 1 - pos
            span = min(127, nhi - pos, k)
            assert span >= 1, f"node {pos} exceeds window cap"
            wins.append((pos, span))
            pos += span
        core_windows.append(wins)

    NW = max(len(w) for w in core_windows)
    T = -(-NW * TPW // GROUP_T) * GROUP_T

    regions = []
    t = 0
    while t < T:
        n = min(24 if t < 48 else REGION_T, T - t)
        regions.append((t, n))
        t += n

    h_f = np.asarray(h, np.float32)
    W1 = np.asarray(W1, np.float32)
    w1a_f = W1[2 * H]
    A_tab = h_f @ W1[:H]
    B_tab8 = (h_f @ W1[H:2 * H]).astype(FP8)

    shared = dict(
        b1=np.asarray(b1, np.float32).reshape(H, 1).copy(),
    )
    sc_iota = np.tile(np.arange(TILE_E, dtype=np.float32).astype(BF16),
                      (128, 1))
    sc_ident = np.eye(128, dtype=np.float32).astype(FP8)
    sc_w2 = np.asarray(W2, np.float32).astype(BF16)
    sc_w3 = (np.asarray(W3, np.float32) * W3SCALE).astype(BF16)
    sc_b2 = np.asarray(b2, np.float32).reshape(H, 1)

    attr_f = np.asarray(edge_attr, np.float32).reshape(-1)
    coord_f = np.asarray(coord, np.float32)
    nmask_f = np.asarray(node_mask, np.float32).reshape(-1)

    in_maps = []
    metas = []
    NS = T * TILE_E
    for c in range(ncores):
        wins = core_windows[c]
        col_s = np.zeros(NS, np.int64)
        real = np.zeros(NS, bool)
        lrow_f = np.full(NS, -1.0, np.float32)
        attr_s = np.zeros(NS, np.float32)
        cdt_s = np.zeros((NS, 3), np.float32)
        A_sb = np.zeros((128, NW * H), np.float32)
        coordm = np.zeros((128, NW, 3), np.float32)
        masks = np.zeros((128, NW, 3), np.float32)

        for w, (base, span) in enumerate(wins):
            s = int(np.searchsorted(row_s_all, base, side="left"))
            e = int(np.searchsorted(row_s_all, base + span, side="left"))
            eids = order[s:e]
            m = len(eids)
            assert m <= CAP
            sl = slice(w * CAP, w * CAP + m)
            col_s[sl] = col[eids]
            real[sl] = True
            lrow_f[sl] = (row[eids] - base).astype(np.float32)
            attr_s[sl] = attr_f[eids]
            cdt_s[sl] = cd[eids]

            A_sb[:span, w * H:(w + 1) * H] = A_tab[base:base + span]
            A_sb[127, w * H:(w + 1) * H] = w1a_f
            coordm[:span, w, :] = (coord_f[base:base + span]
                                   * nmask_f[base:base + span, None])
            masks[:span, w, :] = nmask_f[base:base + span, None] / 65536.0

        bedT = B_tab8[col_s].T.copy()            # [128, NS] fp8
        bedT[:, ~real] = FP8(0.0)
        rtT = np.zeros((128, NS), FP8)
        idx = np.nonzero(real)[0]
        rtT[lrow_f[idx].astype(np.int64), idx] = FP8(1.0)
        rtT[127, :] = attr_s.astype(FP8)

        lrow_b = lrow_f.reshape(T, TILE_E).T.astype(BF16)        # [128, T]
        blob = np.zeros((128, 648 + 2 * T), np.uint8)
        blob[:, 0:256] = sc_iota.view(np.uint8).reshape(128, 256)
        blob[:, 256:384] = sc_ident.view(np.uint8).reshape(128, 128)
        blob[:, 384:640] = sc_w2.view(np.uint8).reshape(128, 256)
        blob[:, 640:642] = sc_w3.view(np.uint8).reshape(128, 2)
        blob[:, 644:648] = sc_b2.view(np.uint8).reshape(128, 4)
        blob[:, 648:] = np.ascontiguousarray(lrow_b).view(np.uint8)
        im = dict(
            bedT=np.ascontiguousarray(bedT),
            rtT=np.ascontiguousarray(rtT),
            A_sb=np.ascontiguousarray(A_sb.astype(FP8)),
            cblob=np.ascontiguousarray(blob),
            cdt=np.ascontiguousarray(
                cdt_s.reshape(T, TILE_E, 3).transpose(1, 0, 2)
                .reshape(128, T * 3).astype(BF16)),
            coordm=np.ascontiguousarray(coordm.reshape(128, NW * 3)),
            masks=np.ascontiguousarray(masks.reshape(128, NW * 3)),
        )
        im.update(shared)
        in_maps.append(im)
        metas.append(dict(wins=wins))

    dims = dict(T=T, NW=NW, regions=regions, N=N)
    return in_maps, metas, dims


# ----------------------------------------------------------------------------
# Bass program
# ----------------------------------------------------------------------------

def build_program(dims):
    from concourse import bass, bacc, tile, mybir

    T, NW = dims["T"], dims["NW"]
    regions = dims["regions"]
    CH_E = CHUNK_T * TILE_E                     # 512
    GR_E = GROUP_T * TILE_E                     # 1536
    RE_MAX = REGION_T * TILE_E                  # 6144
    f32 = mybir.dt.float32
    bf16 = mybir.dt.bfloat16
    fp8 = mybir.dt.float8e4
    n_real_t = NW * TPW

    nc = bacc.Bacc("TRN2", target_bir_lowering=False, debug=False,
                   num_swdge_queues=1, dynamic_dma_scratch_size=16384,
                   detect_race_conditions=bool(os.environ.get("KRACE")))

    def din(name, shape, dt):
        return nc.dram_tensor(name, shape, dt, kind="ExternalInput")

    d_bedT = din("bedT", [128, T * TILE_E], fp8)
    d_rtT = din("rtT", [128, T * TILE_E], fp8)
    d_Asb = din("A_sb", [128, NW * H], fp8)
    d_cblob = din("cblob", [128, 648 + 2 * T], mybir.dt.uint8)
    d_cdt = din("cdt", [128, T, 3], bf16)
    d_coordm = din("coordm", [128, NW * 3], f32)
    d_masks = din("masks", [128, NW * 3], f32)
    d_b1 = din("b1", [H, 1], f32)
    d_out = nc.dram_tensor("out", [128, NW * 3], f32, kind="ExternalOutput")

    SILU = mybir.ActivationFunctionType.Silu
    ABL = set((os.environ.get("KABL") or "").split(","))
    if "noact" in ABL:
        SILU = mybir.ActivationFunctionType.Relu
    AOP = mybir.AluOpType

    def tile_window(t):
        w = t // TPW
        first = (t % TPW == 0)
        last = (t % TPW == TPW - 1) or (t == n_real_t - 1)
        return w, first, last

    with tile.TileContext(nc) as tc:
        with (
            tc.tile_pool(name="const", bufs=1) as cpool,
            tc.tile_pool(name="bed", bufs=3) as bpool,
            tc.tile_pool(name="rtt", bufs=3) as rpool,
            tc.tile_pool(name="otrp", bufs=3) as opool,
            tc.tile_pool(name="x1p", bufs=2) as x1pool,
            tc.tile_pool(name="x2p", bufs=2) as x2pool,
            tc.tile_pool(name="cdsp", bufs=3) as spool,
            tc.tile_pool(name="ps1", bufs=1, space="PSUM") as ps1,
            tc.tile_pool(name="ps2", bufs=1, space="PSUM") as ps2,
            tc.tile_pool(name="psc", bufs=1, space="PSUM") as pscp,
            tc.tile_pool(name="pseg", bufs=1, space="PSUM") as psegp,
        ):
            def load(dram, shape, dt, eng=None):
                t = cpool.tile(shape, dt, tag=f"c_{dram.name}")
                (eng or nc.sync).dma_start(t[:], dram[:])
                return t

            b1 = load(d_b1, [H, 1], f32, eng=nc.scalar)

            # region double-buffered streams
            NREG = len(regions)
            bed_t = {}
            rtt_t = {}

            def load_region(r, split=False):
                t0, nt = regions[r]
                bt = bpool.tile([128, RE_MAX], fp8, tag="bed")
                rt = rpool.tile([128, RE_MAX], fp8, tag="rtt")
                eng2 = nc.scalar if split else nc.sync
                # first region streams in group-sized slices so the first
                # matmuls start as soon as 12 tiles have landed
                step = GROUP_T if r == 0 else nt
                for o in range(0, nt, step):
                    n = min(step, nt - o)
                    esl = slice((t0 + o) * TILE_E, (t0 + o + n) * TILE_E)
                    lsl = slice(o * TILE_E, (o + n) * TILE_E)
                    nc.sync.dma_start(bt[:, lsl], d_bedT[:, esl])
                    eng2.dma_start(rt[:, lsl], d_rtT[:, esl])
                bed_t[r] = bt
                rtt_t[r] = rt

            load_region(0, split=True)
            A_sb = cpool.tile([128, NW * H], fp8, tag="c_A_sb")
            asb_cut = min(8, NW) * H
            nc.sync.dma_start(A_sb[:, 0:asb_cut], d_Asb[:, 0:asb_cut])
            cblob = load(d_cblob, [128, 648 + 2 * T], mybir.dt.uint8)
            iota = cblob[:, 0:256].bitcast(bf16)
            ident = cblob[:, 256:384].bitcast(fp8)
            w2 = cblob[:, 384:640].bitcast(bf16)
            w3 = cblob[:, 640:642].bitcast(bf16)
            b2 = cblob[:, 644:648].bitcast(f32)
            lrow = cblob[:, 648:648 + 2 * T].bitcast(bf16)
            cdt = load(d_cdt, [128, T, 3], bf16)
            if NREG > 1:
                load_region(1, split=True)
            if asb_cut < NW * H:
                nc.sync.dma_start(A_sb[:, asb_cut:], d_Asb[:, asb_cut:])
            # trigger the Silu ACT table load now that all ramp DMA-gen
            # instructions are queued (table DMAs overlap data in flight)
            actwarm = cpool.tile([128, 1], bf16, tag="actwarm")
            nc.vector.memset(actwarm[:], 0.0)
            nc.scalar.activation(actwarm[:], actwarm[:], SILU)
            # keep the PE busy through the DMA ramp so HAM is warm (2.4GHz)
            # when the first real matmuls issue
            pewarm = cpool.tile([128, 128], bf16, tag="pewarm")
            nc.vector.memset(pewarm[:], 0.0)
            pswarm = ps1.tile([128, 512], f32, tag="p1")
            for _ in range(75):
                nc.tensor.matmul(pswarm[:, 0:128], pewarm[:], pewarm[:],
                                 start=True, stop=True,
                                 skip_group_check=True)

            agg = cpool.tile([128, NW * 3], f32, tag="agg")
            nc.vector.memset(agg[:], 0.0)

            pseg_live = [None]
            pending = None

            def emit_tail(g, x1, otr_g):
                t0 = g * GROUP_T
                p2 = ps2.tile([128, GR_E], f32, tag="p2")
                for c in range(GROUP_C):
                    nc.tensor.matmul(p2[:, c * CH_E:(c + 1) * CH_E], w2[:],
                                     x1[:, c * CH_E:(c + 1) * CH_E],
                                     start=True, stop=True,
                                     skip_group_check=True)
                x2 = x2pool.tile([128, GR_E], bf16, tag="x2")
                nc.scalar.activation(x2[:], p2[:], SILU, bias=b2[:])

                psc = pscp.tile([128, GROUP_T], f32, tag="psc")
                for j in range(GROUP_T):
                    if t0 + j >= n_real_t:
                        break
                    nc.tensor.matmul(
                        psc[:, j:j + 1],
                        x2[:, j * TILE_E:(j + 1) * TILE_E],
                        w3[:], start=True, stop=True,
                        skip_group_check=True)
                if "noseg" in ABL:
                    return
                nj = min(GROUP_T, n_real_t - t0)
                cds = spool.tile([128, GROUP_T, 3], fp8, tag="cds")
                nc.vector.tensor_tensor(
                    cds[:, :nj, :], cdt[:, t0:t0 + nj, :],
                    psc[:, :nj, None].broadcast_to([128, nj, 3]),
                    AOP.mult)
                for j in range(nj):
                    gt = t0 + j
                    w, first, last = tile_window(gt)
                    if first:
                        npseg = psegp.tile([128, 3], f32, tag="pseg")
                        pseg_live[0] = npseg
                    ps = pseg_live[0]
                    nc.tensor.matmul(
                        ps[:], otr_g[:, j, :], cds[:, j, :],
                        start=first, stop=last, skip_group_check=True)
                    if last:
                        sl = agg[:, w * 3:w * 3 + 3]
                        nc.vector.tensor_add(sl, sl, ps[:])

            NGRP = T // GROUP_T
            reg_of = {}
            for ri, (rt, rn) in enumerate(regions):
                for tt in range(rt, rt + rn):
                    reg_of[tt] = ri
            for g in range(NGRP):
                t0 = g * GROUP_T
                r = reg_of[t0]
                rt0 = regions[r][0]
                if t0 >= n_real_t:
                    break
                if t0 == rt0 and r + 2 <= NREG - 1:
                    load_region(r + 2)
                off = (t0 - rt0) * TILE_E

                # one-hot [e, slot] for this group's seg matmuls (DVE)
                otr_g = opool.tile([128, GROUP_T, TILE_E], fp8, tag="otr")
                nc.vector.tensor_tensor(
                    otr_g[:],
                    iota[:, None, :].broadcast_to([128, GROUP_T, TILE_E]),
                    lrow[:, t0:t0 + GROUP_T, None].broadcast_to(
                        [128, GROUP_T, TILE_E]),
                    AOP.is_equal)

                p1 = ps1.tile([128, GR_E], f32, tag="p1")
                bed = bed_t[r]
                rtt = rtt_t[r]
                # B-insert opens each bank; A-select accumulates and closes
                for c in range(GROUP_C):
                    co = c * CH_E
                    nc.tensor.matmul(
                        p1[:, co:co + CH_E], ident[:],
                        bed[:, off + co:off + co + CH_E],
                        start=True, stop=False, skip_group_check=True)
                for c in range(GROUP_C):
                    tc0 = t0 + c * CHUNK_T
                    runs = []
                    for t in range(CHUNK_T):
                        gt = tc0 + t
                        w = 0 if gt >= n_real_t else gt // TPW
                        if runs and runs[-1][0] == w:
                            runs[-1][2] = t + 1
                        else:
                            runs.append([w, t, t + 1])
                    co = c * CH_E
                    for w, ta, tb in runs:
                        nc.tensor.matmul(
                            p1[:, co + ta * TILE_E:co + tb * TILE_E],
                            A_sb[:, w * H:(w + 1) * H],
                            rtt[:, off + co + ta * TILE_E:
                                   off + co + tb * TILE_E],
                            start=False, stop=True, skip_group_check=True)

                x1 = x1pool.tile([128, GR_E], bf16, tag="x1")
                nc.scalar.activation(x1[:], p1[:], SILU, bias=b1[:])

                if pending is not None:
                    emit_tail(*pending)
                pending = (g, x1, otr_g)

            coordm = load(d_coordm, [128, NW * 3], f32)
            masks = load(d_masks, [128, NW * 3], f32)
            if pending is not None:
                emit_tail(*pending)

            outs = cpool.tile([128, NW * 3], f32, tag="outs")
            nc.vector.tensor_mul(outs[:], agg[:], masks[:])
            nc.vector.tensor_add(outs[:], outs[:], coordm[:])
            nc.sync.dma_start(d_out[:], outs[:])

    nc.compile()
    return nc


# ----------------------------------------------------------------------------
# Entry point
# ----------------------------------------------------------------------------

LAST_RESULTS = None


def _ensure_ntff_hook():
    """Register the axon NTFF profile hook if the image lacks antenv.axon_hooks."""
    import types
    try:
        from antenv.axon_hooks import get_axon_ntff_profile_hook  # noqa: F401
        return
    except ImportError:
        pass
    holder = {}
    mod = types.ModuleType("antenv.axon_hooks")
    mod.set_axon_ntff_profile_hook = lambda h: holder.__setitem__("h", h)
    mod.get_axon_ntff_profile_hook = lambda: holder.get("h")
    sys.modules["antenv.axon_hooks"] = mod
    try:
        sys.path.insert(0, "/root/.axon_site")
        from trn_agent_boot.trn_boot import _ntff_profile_via_ctypes
        hook = _ntff_profile_via_ctypes("/opt/axon/libaxon_pjrt.so")
        if hook is not None:
            mod.set_axon_ntff_profile_hook(hook)
    except Exception as e:  # degrade to no trace
        print("ntff hook setup failed:", e)
    from concourse import bass_utils as _bu
    _bu.upload_artifacts = lambda tmpdir: f"local:{tmpdir}"


def kernel(**inputs):
    global LAST_RESULTS
    from concourse.bass_utils import run_bass_kernel_spmd

    in_maps, metas, dims = prep_host(**inputs)
    nc = build_program(dims)
    trace = bool(os.environ.get("KERNEL_TRACE"))
    if trace:
        _ensure_ntff_hook()
    tmpdir = os.environ.get("KERNEL_TRACE_DIR") or None
    res = run_bass_kernel_spmd(nc, in_maps, list(range(NCORES)), trace=trace,
                               tmpdir=tmpdir)
    LAST_RESULTS = res

    N = dims["N"]
    NW = dims["NW"]
    out = np.zeros((N, 3), np.float32)
    for c in range(NCORES):
        o = res.results[c]["out"].reshape(128, NW, 3)
        for w, (base, span) in enumerate(metas[c]["wins"]):
            out[base:base + span] = o[:span, w, :]
    return out


# revision 35
# speedup vs baseline: 1.0125x; 1.0107x over previous
"""Trainium2 Bass kernel for nn_EquivariantUpdate (GNN message passing).

Strategy (edge-parallel across 8 NeuronCores, SPMD single program):
  - Host splits nodes into 8 contiguous ranges balanced by edge count; core c
    owns its node range and all edges whose row falls in it, so the
    segment-sum is core-local (no collectives).
  - Host precomputes the node-level tables A = h@W1[:128] and
    B = h@W1[128:256] (as in the prior baseline) and marshals all per-edge
    data into a windowed layout: nodes pack into variable-span windows
    (<=127 nodes, TPW=17 tiles of 128 edge slots each); every window gets
    exactly TPW tiles, zero-padded.  Per-edge tensors are laid out
    [feature/slot, edge] so the device consumes them directly:
      * bedT  fp8 [128, T*128]  = B[col_e] per edge slot (feature-major)
      * rtT   fp8 [128, T*128]  = one-hot(local row) with row 127 = attr_e
      * A_sb  fp8 [128, NW*128] = window A slices, slot 127 = W1 attr row
      * cdt   bf16 [128, T, 3]  = masked coord_diff, lrow bf16 [128, T]
  - Device per 512-edge chunk: p1(psum) = A_sb-window^T @ rtT (adds
    A[row]+attr*w1a) + I @ bedT (adds B[col]); both silu layers run on the
    ACT engine at 1536-wide calls in a one-group-lag software pipeline so
    ACT (the wall) never idles; W2 via fp8 matmul; scale = x2-tile^T @ w3
    (fp8, 4 small MMs/chunk); cds = cdt * psc on DVE; segment-sum via
    per-tile one-hot matmuls (otr built on DVE from lrow+iota) accumulated
    in a [128,3] psum per window.
  - w3 is prescaled by 2**16/100 on host (fp8 range); the final output
    stage computes out = agg * (mask/2**16) + coord*mask and the host
    reassembles the variable window spans.
"""

import sys
import os

sys.path.insert(0, "/opt/trn_rl_repo")

import numpy as np
import ml_dtypes

BF16 = ml_dtypes.bfloat16
FP8 = ml_dtypes.float8_e4m3fn

H = 128
NCORES = 8
TILE_E = 128
CHUNK_T = 4          # tiles per MLP chunk (512 edges)
GROUP_C = 3          # chunks per ACT group (1536 edges, 3 psum banks)
GROUP_T = CHUNK_T * GROUP_C          # 16 tiles per group == one window
REGION_T = 48        # tiles per DMA region (3 groups)
TPW = 16             # tiles per window (cap 2048 edges, chunk-aligned)
NORM = 100.0
W3SCALE = 65536.0 / NORM


# ----------------------------------------------------------------------------
# Host-side preparation
# ----------------------------------------------------------------------------

def prep_host(h, coord, edge_index, coord_diff, edge_attr, node_mask,
              edge_mask, W1, b1, W2, b2, W3, ncores=NCORES):
    N = h.shape[0]
    E = edge_index.shape[1]
    row = np.asarray(edge_index[0], dtype=np.int64)
    col = np.asarray(edge_index[1], dtype=np.int64)
    cd = (np.asarray(coord_diff, np.float32)
          * np.asarray(edge_mask, np.float32))          # fold edge_mask

    counts = np.bincount(row, minlength=N)
    cum = np.cumsum(counts)
    bounds = [0]
    for c in range(1, ncores):
        bounds.append(int(np.searchsorted(cum, c * E / ncores)))
    bounds.append(N)

    order = np.argsort(row, kind="stable")
    row_s_all = row[order]

    CAP = TPW * TILE_E
    cum0 = np.concatenate([[0], cum])

    core_windows = []
    for c in range(ncores):
        nlo, nhi = bounds[c], bounds[c + 1]
        wins = []
        pos = nlo
        while pos < nhi:
            k = int(np.searchsorted(cum0, cum0[pos] + CAP,
                                    side="right")) - 1 - pos
            span = min(127, nhi - pos, k)
            assert span >= 1, f"node {pos} exceeds window cap"
            wins.append((pos, span))
            pos += span
        core_windows.append(wins)

    NW = max(len(w) for w in core_windows)
    T = -(-NW * TPW // GROUP_T) * GROUP_T

    regions = []
    t = 0
    while t < T:
        n = min(24 if t < 48 else REGION_T, T - t)
        regions.append((t, n))
        t += n

    h_f = np.asarray(h, np.float32)
    W1 = np.asarray(W1, np.float32)
    w1a_f = W1[2 * H]
    A_tab = h_f @ W1[:H]
    B_tab8 = (h_f @ W1[H:2 * H]).astype(FP8)

    shared = dict(
        b1=np.asarray(b1, np.float32).reshape(H, 1).copy(),
    )
    sc_iota = np.tile(np.arange(TILE_E, dtype=np.float32).astype(BF16),
                      (128, 1))
    sc_ident = np.eye(128, dtype=np.float32).astype(FP8)
    sc_w2 = np.asarray(W2, np.float32).astype(BF16)
    sc_w3 = (np.asarray(W3, np.float32) * W3SCALE).astype(BF16)
    sc_b2 = np.asarray(b2, np.float32).reshape(H, 1)

    attr_f = np.asarray(edge_attr, np.float32).reshape(-1)
    coord_f = np.asarray(coord, np.float32)
    nmask_f = np.asarray(node_mask, np.float32).reshape(-1)

    in_maps = []
    metas = []
    NS = T * TILE_E
    for c in range(ncores):
        wins = core_windows[c]
        col_s = np.zeros(NS, np.int64)
        real = np.zeros(NS, bool)
        lrow_f = np.full(NS, -1.0, np.float32)
        attr_s = np.zeros(NS, np.float32)
        cdt_s = np.zeros((NS, 3), np.float32)
        A_sb = np.zeros((128, NW * H), np.float32)
        coordm = np.zeros((128, NW, 3), np.float32)
        masks = np.zeros((128, NW, 3), np.float32)

        for w, (base, span) in enumerate(wins):
            s = int(np.searchsorted(row_s_all, base, side="left"))
            e = int(np.searchsorted(row_s_all, base + span, side="left"))
            eids = order[s:e]
            m = len(eids)
            assert m <= CAP
            sl = slice(w * CAP, w * CAP + m)
            col_s[sl] = col[eids]
            real[sl] = True
            lrow_f[sl] = (row[eids] - base).astype(np.float32)
            attr_s[sl] = attr_f[eids]
            cdt_s[sl] = cd[eids]

            A_sb[:span, w * H:(w + 1) * H] = A_tab[base:base + span]
            A_sb[127, w * H:(w + 1) * H] = w1a_f
            coordm[:span, w, :] = (coord_f[base:base + span]
                                   * nmask_f[base:base + span, None])
            masks[:span, w, :] = nmask_f[base:base + span, None] / 65536.0

        bedT = B_tab8[col_s].T.copy()            # [128, NS] fp8
        bedT[:, ~real] = FP8(0.0)
        rtT = np.zeros((128, NS), FP8)
        idx = np.nonzero(real)[0]
        rtT[lrow_f[idx].astype(np.int64), idx] = FP8(1.0)
        rtT[127, :] = attr_s.astype(FP8)

        lrow_b = lrow_f.reshape(T, TILE_E).T.astype(BF16)        # [128, T]
        blob = np.zeros((128, 648 + 2 * T), np.uint8)
        blob[:, 0:256] = sc_iota.view(np.uint8).reshape(128, 256)
        blob[:, 256:384] = sc_ident.view(np.uint8).reshape(128, 128)
        blob[:, 384:640] = sc_w2.view(np.uint8).reshape(128, 256)
        blob[:, 640:642] = sc_w3.view(np.uint8).reshape(128, 2)
        blob[:, 644:648] = sc_b2.view(np.uint8).reshape(128, 4)
        blob[:, 648:] = np.ascontiguousarray(lrow_b).view(np.uint8)
        im = dict(
            bedT=np.ascontiguousarray(bedT),
            rtT=np.ascontiguousarray(rtT),
            A_sb=np.ascontiguousarray(A_sb.astype(FP8)),
            cblob=np.ascontiguousarray(blob),
            cdt=np.ascontiguousarray(
                cdt_s.reshape(T, TILE_E, 3).transpose(1, 0, 2)
                .reshape(128, T * 3).astype(BF16)),
            coordm=np.ascontiguousarray(coordm.reshape(128, NW * 3)),
            masks=np.ascontiguousarray(masks.reshape(128, NW * 3)),
        )
        im.update(shared)
        in_maps.append(im)
        metas.append(dict(wins=wins))

    dims = dict(T=T, NW=NW, regions=regions, N=N)
    return in_maps, metas, dims


# ----------------------------------------------------------------------------
# Bass program
# ----------------------------------------------------------------------------

def build_program(dims):
    from concourse import bass, bacc, tile, mybir

    T, NW = dims["T"], dims["NW"]
    regions = dims["regions"]
    CH_E = CHUNK_T * TILE_E                     # 512
    GR_E = GROUP_T * TILE_E                     # 1536
    RE_MAX = REGION_T * TILE_E                  # 6144
    f32 = mybir.dt.float32
    bf16 = mybir.dt.bfloat16
    fp8 = mybir.dt.float8e4
    n_real_t = NW * TPW

    nc = bacc.Bacc("TRN2", target_bir_lowering=False, debug=False,
                   num_swdge_queues=1, dynamic_dma_scratch_size=16384,
                   detect_race_conditions=bool(os.environ.get("KRACE")))

    def din(name, shape, dt):
        return nc.dram_tensor(name, shape, dt, kind="ExternalInput")

    d_bedT = din("bedT", [128, T * TILE_E], fp8)
    d_rtT = din("rtT", [128, T * TILE_E], fp8)
    d_Asb = din("A_sb", [128, NW * H], fp8)
    d_cblob = din("cblob", [128, 648 + 2 * T], mybir.dt.uint8)
    d_cdt = din("cdt", [128, T, 3], bf16)
    d_coordm = din("coordm", [128, NW * 3], f32)
    d_masks = din("masks", [128, NW * 3], f32)
    d_b1 = din("b1", [H, 1], f32)
    d_out = nc.dram_tensor("out", [128, NW * 3], f32, kind="ExternalOutput")

    SILU = mybir.ActivationFunctionType.Silu
    ABL = set((os.environ.get("KABL") or "").split(","))
    if "noact" in ABL:
        SILU = mybir.ActivationFunctionType.Relu
    AOP = mybir.AluOpType

    def tile_window(t):
        w = t // TPW
        first = (t % TPW == 0)
        last = (t % TPW == TPW - 1) or (t == n_real_t - 1)
        return w, first, last

    with tile.TileContext(nc) as tc:
        with (
            tc.tile_pool(name="const", bufs=1) as cpool,
            tc.tile_pool(name="bed", bufs=3) as bpool,
            tc.tile_pool(name="rtt", bufs=3) as rpool,
            tc.tile_pool(name="otrp", bufs=3) as opool,
            tc.tile_pool(name="x1p", bufs=2) as x1pool,
            tc.tile_pool(name="x2p", bufs=2) as x2pool,
            tc.tile_pool(name="cdsp", bufs=3) as spool,
            tc.tile_pool(name="ps1", bufs=1, space="PSUM") as ps1,
            tc.tile_pool(name="ps2", bufs=1, space="PSUM") as ps2,
            tc.tile_pool(name="psc", bufs=1, space="PSUM") as pscp,
            tc.tile_pool(name="pseg", bufs=1, space="PSUM") as psegp,
        ):
            def load(dram, shape, dt, eng=None):
                t = cpool.tile(shape, dt, tag=f"c_{dram.name}")
                (eng or nc.sync).dma_start(t[:], dram[:])
                return t

            b1 = load(d_b1, [H, 1], f32, eng=nc.scalar)

            # region double-buffered streams
            NREG = len(regions)
            bed_t = {}
            rtt_t = {}

            def load_region(r, split=False):
                t0, nt = regions[r]
                bt = bpool.tile([128, RE_MAX], fp8, tag="bed")
                rt = rpool.tile([128, RE_MAX], fp8, tag="rtt")
                eng2 = nc.scalar if split else nc.sync
                # first region streams in group-sized slices so the first
                # matmuls start as soon as 12 tiles have landed
                step = GROUP_T if r == 0 else nt
                for o in range(0, nt, step):
                    n = min(step, nt - o)
                    esl = slice((t0 + o) * TILE_E, (t0 + o + n) * TILE_E)
                    lsl = slice(o * TILE_E, (o + n) * TILE_E)
                    nc.sync.dma_start(bt[:, lsl], d_bedT[:, esl])
                    eng2.dma_start(rt[:, lsl], d_rtT[:, esl])
                bed_t[r] = bt
                rtt_t[r] = rt

            load_region(0, split=True)
            A_sb = cpool.tile([128, NW * H], fp8, tag="c_A_sb")
            asb_cut = min(8, NW) * H
            nc.sync.dma_start(A_sb[:, 0:asb_cut], d_Asb[:, 0:asb_cut])
            cblob = load(d_cblob, [128, 648 + 2 * T], mybir.dt.uint8)
            iota = cblob[:, 0:256].bitcast(bf16)
            ident = cblob[:, 256:384].bitcast(fp8)
            w2 = cblob[:, 384:640].bitcast(bf16)
            w3 = cblob[:, 640:642].bitcast(bf16)
            b2 = cblob[:, 644:648].bitcast(f32)
            lrow = cblob[:, 648:648 + 2 * T].bitcast(bf16)
            cdt = load(d_cdt, [128, T, 3], bf16)
            if NREG > 1:
                load_region(1, split=True)
            if asb_cut < NW * H:
                nc.sync.dma_start(A_sb[:, asb_cut:], d_Asb[:, asb_cut:])
            # trigger the Silu ACT table load now that all ramp DMA-gen
            # instructions are queued (table DMAs overlap data in flight)
            actwarm = cpool.tile([128, 1], bf16, tag="actwarm")
            nc.vector.memset(actwarm[:], 0.0)
            nc.scalar.activation(actwarm[:], actwarm[:], SILU)
            # keep the PE busy through the DMA ramp so HAM is warm (2.4GHz)
            # when the first real matmuls issue
            pewarm = cpool.tile([128, 128], bf16, tag="pewarm")
            nc.vector.memset(pewarm[:], 0.0)
            pswarm = ps1.tile([128, 512], f32, tag="p1")
            for _ in range(75):
                nc.tensor.matmul(pswarm[:, 0:128], pewarm[:], pewarm[:],
                                 start=True, stop=True,
                                 skip_group_check=True)

            agg = cpool.tile([128, NW * 3], f32, tag="agg")
            nc.vector.memset(agg[:], 0.0)

            pseg_live = [None]
            pending = None

            def emit_tail(g, x1, otr_g):
                t0 = g * GROUP_T
                p2 = ps2.tile([128, GR_E], f32, tag="p2")
                for c in range(GROUP_C):
                    nc.tensor.matmul(p2[:, c * CH_E:(c + 1) * CH_E], w2[:],
                                     x1[:, c * CH_E:(c + 1) * CH_E],
                                     start=True, stop=True,
                                     skip_group_check=True)
                x2 = x2pool.tile([128, GR_E], bf16, tag="x2")
                nc.scalar.activation(x2[:], p2[:], SILU, bias=b2[:])

                psc = pscp.tile([128, GROUP_T], f32, tag="psc")
                for j in range(GROUP_T):
                    if t0 + j >= n_real_t:
                        break
                    nc.tensor.matmul(
                        psc[:, j:j + 1],
                        x2[:, j * TILE_E:(j + 1) * TILE_E],
                        w3[:], start=True, stop=True,
                        skip_group_check=True)
                if "noseg" in ABL:
                    return
                nj = min(GROUP_T, n_real_t - t0)
                cds = spool.tile([128, GROUP_T, 3], fp8, tag="cds")
                nc.vector.tensor_tensor(
                    cds[:, :nj, :], cdt[:, t0:t0 + nj, :],
                    psc[:, :nj, None].broadcast_to([128, nj, 3]),
                    AOP.mult)
                for j in range(nj):
                    gt = t0 + j
                    w, first, last = tile_window(gt)
                    if first:
                        npseg = psegp.tile([128, 3], f32, tag="pseg")
                        pseg_live[0] = npseg
                    ps = pseg_live[0]
                    nc.tensor.matmul(
                        ps[:], otr_g[:, j, :], cds[:, j, :],
                        start=first, stop=last, skip_group_check=True)
                    if last:
                        sl = agg[:, w * 3:w * 3 + 3]
                        nc.vector.tensor_add(sl, sl, ps[:])

            NGRP = T // GROUP_T
            reg_of = {}
            for ri, (rt, rn) in enumerate(regions):
                for tt in range(rt, rt + rn):
                    reg_of[tt] = ri
            for g in range(NGRP):
                t0 = g * GROUP_T
                r = reg_of[t0]
                rt0 = regions[r][0]
                if t0 >= n_real_t:
                    break
                if t0 == rt0 and r + 2 <= NREG - 1:
                    load_region(r + 2)
                off = (t0 - rt0) * TILE_E

                # one-hot [e, slot] for this group's seg matmuls (DVE)
                otr_g = opool.tile([128, GROUP_T, TILE_E], fp8, tag="otr")
                nc.vector.tensor_tensor(
                    otr_g[:],
                    iota[:, None, :].broadcast_to([128, GROUP_T, TILE_E]),
                    lrow[:, t0:t0 + GROUP_T, None].broadcast_to(
                        [128, GROUP_T, TILE_E]),
                    AOP.is_equal)

                p1 = ps1.tile([128, GR_E], f32, tag="p1")
                bed = bed_t[r]
                rtt = rtt_t[r]
                # B-insert opens each bank; A-select accumulates and closes
                for c in range(GROUP_C):
                    co = c * CH_E
                    nc.tensor.matmul(
                        p1[:, co:co + CH_E], ident[:],
                        bed[:, off + co:off + co + CH_E],
                        start=True, stop=False, skip_group_check=True)
                for c in range(GROUP_C):
                    tc0 = t0 + c * CHUNK_T
                    runs = []
                    for t in range(CHUNK_T):
                        gt = tc0 + t
                        w = 0 if gt >= n_real_t else gt // TPW
                        if runs and runs[-1][0] == w:
                            runs[-1][2] = t + 1
                        else:
                            runs.append([w, t, t + 1])
                    co = c * CH_E
                    for w, ta, tb in runs:
                        nc.tensor.matmul(
                            p1[:, co + ta * TILE_E:co + tb * TILE_E],
                            A_sb[:, w * H:(w + 1) * H],
                            rtt[:, off + co + ta * TILE_E:
                                   off + co + tb * TILE_E],
                            start=False, stop=True, skip_group_check=True)

                x1 = x1pool.tile([128, GR_E], bf16, tag="x1")
                nc.scalar.activation(x1[:], p1[:], SILU, bias=b1[:])

                if pending is not None:
                    emit_tail(*pending)
                pending = (g, x1, otr_g)

            coordm = load(d_coordm, [128, NW * 3], f32)
            masks = load(d_masks, [128, NW * 3], f32)
            if pending is not None:
                emit_tail(*pending)

            outs = cpool.tile([128, NW * 3], f32, tag="outs")
            nc.vector.tensor_mul(outs[:], agg[:], masks[:])
            nc.vector.tensor_add(outs[:], outs[:], coordm[:])
            nc.sync.dma_start(d_out[:], outs[:])

    nc.compile()
    return nc


# ----------------------------------------------------------------------------
# Entry point
# ----------------------------------------------------------------------------

LAST_RESULTS = None


def _ensure_ntff_hook():
    """Register the axon NTFF profile hook if the image lacks antenv.axon_hooks."""
    import types
    try:
        from antenv.axon_hooks import get_axon_ntff_profile_hook  # noqa: F401
        return
    except ImportError:
        pass
    holder = {}
    mod = types.ModuleType("antenv.axon_hooks")
    mod.set_axon_ntff_profile_hook = lambda h: holder.__setitem__("h", h)
    mod.get_axon_ntff_profile_hook = lambda: holder.get("h")
    sys.modules["antenv.axon_hooks"] = mod
    try:
        sys.path.insert(0, "/root/.axon_site")
        from trn_agent_boot.trn_boot import _ntff_profile_via_ctypes
        hook = _ntff_profile_via_ctypes("/opt/axon/libaxon_pjrt.so")
        if hook is not None:
            mod.set_axon_ntff_profile_hook(hook)
    except Exception as e:  # degrade to no trace
        print("ntff hook setup failed:", e)
    from concourse import bass_utils as _bu
    _bu.upload_artifacts = lambda tmpdir: f"local:{tmpdir}"


def kernel(**inputs):
    global LAST_RESULTS
    from concourse.bass_utils import run_bass_kernel_spmd

    in_maps, metas, dims = prep_host(**inputs)
    nc = build_program(dims)
    trace = bool(os.environ.get("KERNEL_TRACE"))
    if trace:
        _ensure_ntff_hook()
    tmpdir = os.environ.get("KERNEL_TRACE_DIR") or None
    res = run_bass_kernel_spmd(nc, in_maps, list(range(NCORES)), trace=trace,
                               tmpdir=tmpdir)
    LAST_RESULTS = res

    N = dims["N"]
    NW = dims["NW"]
    out = np.zeros((N, 3), np.float32)
    for c in range(NCORES):
        o = res.results[c]["out"].reshape(128, NW, 3)
        for w, (base, span) in enumerate(metas[c]["wins"]):
            out[base:base + span] = o[:span, w, :]
    return out


# revision 36
# speedup vs baseline: 1.0194x; 1.0068x over previous
"""Trainium2 Bass kernel for nn_EquivariantUpdate (GNN message passing).

Strategy (edge-parallel across 8 NeuronCores, SPMD single program):
  - Host splits nodes into 8 contiguous ranges balanced by edge count; core c
    owns its node range and all edges whose row falls in it, so the
    segment-sum is core-local (no collectives).
  - Host precomputes the node-level tables A = h@W1[:128] and
    B = h@W1[128:256] (as in the prior baseline) and marshals all per-edge
    data into a windowed layout: nodes pack into variable-span windows
    (<=127 nodes, TPW=17 tiles of 128 edge slots each); every window gets
    exactly TPW tiles, zero-padded.  Per-edge tensors are laid out
    [feature/slot, edge] so the device consumes them directly:
      * bedT  fp8 [128, T*128]  = B[col_e] per edge slot (feature-major)
      * rtT   fp8 [128, T*128]  = one-hot(local row) with row 127 = attr_e
      * A_sb  fp8 [128, NW*128] = window A slices, slot 127 = W1 attr row
      * cdt   bf16 [128, T, 3]  = masked coord_diff, lrow bf16 [128, T]
  - Device per 512-edge chunk: p1(psum) = A_sb-window^T @ rtT (adds
    A[row]+attr*w1a) + I @ bedT (adds B[col]); both silu layers run on the
    ACT engine at 1536-wide calls in a one-group-lag software pipeline so
    ACT (the wall) never idles; W2 via fp8 matmul; scale = x2-tile^T @ w3
    (fp8, 4 small MMs/chunk); cds = cdt * psc on DVE; segment-sum via
    per-tile one-hot matmuls (otr built on DVE from lrow+iota) accumulated
    in a [128,3] psum per window.
  - w3 is prescaled by 2**16/100 on host (fp8 range); the final output
    stage computes out = agg * (mask/2**16) + coord*mask and the host
    reassembles the variable window spans.
"""

import sys
import os

sys.path.insert(0, "/opt/trn_rl_repo")

import numpy as np
import ml_dtypes

BF16 = ml_dtypes.bfloat16
FP8 = ml_dtypes.float8_e4m3fn

H = 128
NCORES = 8
TILE_E = 128
CHUNK_T = 4          # tiles per MLP chunk (512 edges)
GROUP_C = 3          # chunks per ACT group (1536 edges, 3 psum banks)
GROUP_T = CHUNK_T * GROUP_C          # 16 tiles per group == one window
REGION_T = 48        # tiles per DMA region (3 groups)
TPW = 16             # tiles per window (cap 2048 edges, chunk-aligned)
NORM = 100.0
W3SCALE = 65536.0 / NORM


# ----------------------------------------------------------------------------
# Host-side preparation
# ----------------------------------------------------------------------------

def prep_host(h, coord, edge_index, coord_diff, edge_attr, node_mask,
              edge_mask, W1, b1, W2, b2, W3, ncores=NCORES):
    N = h.shape[0]
    E = edge_index.shape[1]
    row = np.asarray(edge_index[0], dtype=np.int64)
    col = np.asarray(edge_index[1], dtype=np.int64)
    cd = (np.asarray(coord_diff, np.float32)
          * np.asarray(edge_mask, np.float32))          # fold edge_mask

    counts = np.bincount(row, minlength=N)
    cum = np.cumsum(counts)
    bounds = [0]
    for c in range(1, ncores):
        bounds.append(int(np.searchsorted(cum, c * E / ncores)))
    bounds.append(N)

    order = np.argsort(row, kind="stable")
    row_s_all = row[order]

    CAP = TPW * TILE_E
    cum0 = np.concatenate([[0], cum])

    core_windows = []
    for c in range(ncores):
        nlo, nhi = bounds[c], bounds[c + 1]
        wins = []
        pos = nlo
        while pos < nhi:
            k = int(np.searchsorted(cum0, cum0[pos] + CAP,
                                    side="right")) - 1 - pos
            span = min(127, nhi - pos, k)
            assert span >= 1, f"node {pos} exceeds window cap"
            wins.append((pos, span))
            pos += span
        core_windows.append(wins)

    NW = max(len(w) for w in core_windows)
    T = -(-NW * TPW // GROUP_T) * GROUP_T

    regions = []
    t = 0
    while t < T:
        n = min(24 if t < 48 else REGION_T, T - t)
        regions.append((t, n))
        t += n

    h_f = np.asarray(h, np.float32)
    W1 = np.asarray(W1, np.float32)
    w1a_f = W1[2 * H]
    A_tab = h_f @ W1[:H]
    B_tab8 = (h_f @ W1[H:2 * H]).astype(FP8)

    shared = dict(
        b1=np.asarray(b1, np.float32).reshape(H, 1).copy(),
    )
    sc_iota = np.tile(np.arange(TILE_E, dtype=np.float32).astype(BF16),
                      (128, 1))
    sc_ident = np.eye(128, dtype=np.float32).astype(FP8)
    sc_w2 = np.asarray(W2, np.float32).astype(BF16)
    sc_w3 = (np.asarray(W3, np.float32) * W3SCALE).astype(BF16)
    sc_b2 = np.asarray(b2, np.float32).reshape(H, 1)

    attr_f = np.asarray(edge_attr, np.float32).reshape(-1)
    coord_f = np.asarray(coord, np.float32)
    nmask_f = np.asarray(node_mask, np.float32).reshape(-1)

    in_maps = []
    metas = []
    NS = T * TILE_E
    for c in range(ncores):
        wins = core_windows[c]
        col_s = np.zeros(NS, np.int64)
        real = np.zeros(NS, bool)
        lrow_f = np.full(NS, -1.0, np.float32)
        attr_s = np.zeros(NS, np.float32)
        cdt_s = np.zeros((NS, 3), np.float32)
        A_sb = np.zeros((128, NW * H), np.float32)
        coordm = np.zeros((128, NW, 3), np.float32)
        masks = np.zeros((128, NW, 3), np.float32)

        for w, (base, span) in enumerate(wins):
            s = int(np.searchsorted(row_s_all, base, side="left"))
            e = int(np.searchsorted(row_s_all, base + span, side="left"))
            eids = order[s:e]
            m = len(eids)
            assert m <= CAP
            sl = slice(w * CAP, w * CAP + m)
            col_s[sl] = col[eids]
            real[sl] = True
            lrow_f[sl] = (row[eids] - base).astype(np.float32)
            attr_s[sl] = attr_f[eids]
            cdt_s[sl] = cd[eids]

            A_sb[:span, w * H:(w + 1) * H] = A_tab[base:base + span]
            A_sb[127, w * H:(w + 1) * H] = w1a_f
            coordm[:span, w, :] = (coord_f[base:base + span]
                                   * nmask_f[base:base + span, None])
            masks[:span, w, :] = nmask_f[base:base + span, None] / 65536.0

        bedT = B_tab8[col_s].T.copy()            # [128, NS] fp8
        bedT[:, ~real] = FP8(0.0)
        rtT = np.zeros((128, NS), FP8)
        idx = np.nonzero(real)[0]
        rtT[lrow_f[idx].astype(np.int64), idx] = FP8(1.0)
        rtT[127, :] = attr_s.astype(FP8)

        lrow_b = lrow_f.reshape(T, TILE_E).T.astype(BF16)        # [128, T]
        blob = np.zeros((128, 648 + 2 * T), np.uint8)
        blob[:, 0:256] = sc_iota.view(np.uint8).reshape(128, 256)
        blob[:, 256:384] = sc_ident.view(np.uint8).reshape(128, 128)
        blob[:, 384:640] = sc_w2.view(np.uint8).reshape(128, 256)
        blob[:, 640:642] = sc_w3.view(np.uint8).reshape(128, 2)
        blob[:, 644:648] = sc_b2.view(np.uint8).reshape(128, 4)
        blob[:, 648:] = np.ascontiguousarray(lrow_b).view(np.uint8)
        im = dict(
            bedT=np.ascontiguousarray(bedT),
            rtT=np.ascontiguousarray(rtT),
            A_sb=np.ascontiguousarray(A_sb.astype(FP8)),
            cblob=np.ascontiguousarray(blob),
            cdt=np.ascontiguousarray(
                cdt_s.reshape(T, TILE_E, 3).transpose(1, 0, 2)
                .reshape(128, T * 3).astype(BF16)),
            coordm=np.ascontiguousarray(coordm.reshape(128, NW * 3)),
            masks=np.ascontiguousarray(masks.reshape(128, NW * 3)),
        )
        im.update(shared)
        in_maps.append(im)
        metas.append(dict(wins=wins))

    dims = dict(T=T, NW=NW, regions=regions, N=N)
    return in_maps, metas, dims


# ----------------------------------------------------------------------------
# Bass program
# ----------------------------------------------------------------------------

def build_program(dims):
    from concourse import bass, bacc, tile, mybir

    T, NW = dims["T"], dims["NW"]
    regions = dims["regions"]
    CH_E = CHUNK_T * TILE_E                     # 512
    GR_E = GROUP_T * TILE_E                     # 1536
    RE_MAX = REGION_T * TILE_E                  # 6144
    f32 = mybir.dt.float32
    bf16 = mybir.dt.bfloat16
    fp8 = mybir.dt.float8e4
    n_real_t = NW * TPW

    nc = bacc.Bacc("TRN2", target_bir_lowering=False, debug=False,
                   num_swdge_queues=1, dynamic_dma_scratch_size=16384,
                   detect_race_conditions=bool(os.environ.get("KRACE")))

    def din(name, shape, dt):
        return nc.dram_tensor(name, shape, dt, kind="ExternalInput")

    d_bedT = din("bedT", [128, T * TILE_E], fp8)
    d_rtT = din("rtT", [128, T * TILE_E], fp8)
    d_Asb = din("A_sb", [128, NW * H], fp8)
    d_cblob = din("cblob", [128, 648 + 2 * T], mybir.dt.uint8)
    d_cdt = din("cdt", [128, T, 3], bf16)
    d_coordm = din("coordm", [128, NW * 3], f32)
    d_masks = din("masks", [128, NW * 3], f32)
    d_b1 = din("b1", [H, 1], f32)
    d_out = nc.dram_tensor("out", [128, NW * 3], f32, kind="ExternalOutput")

    SILU = mybir.ActivationFunctionType.Silu
    ABL = set((os.environ.get("KABL") or "").split(","))
    if "noact" in ABL:
        SILU = mybir.ActivationFunctionType.Relu
    AOP = mybir.AluOpType

    def tile_window(t):
        w = t // TPW
        first = (t % TPW == 0)
        last = (t % TPW == TPW - 1) or (t == n_real_t - 1)
        return w, first, last

    with tile.TileContext(nc) as tc:
        with (
            tc.tile_pool(name="const", bufs=1) as cpool,
            tc.tile_pool(name="bed", bufs=3) as bpool,
            tc.tile_pool(name="rtt", bufs=3) as rpool,
            tc.tile_pool(name="otrp", bufs=3) as opool,
            tc.tile_pool(name="x1p", bufs=2) as x1pool,
            tc.tile_pool(name="x2p", bufs=2) as x2pool,
            tc.tile_pool(name="cdsp", bufs=3) as spool,
            tc.tile_pool(name="ps1", bufs=1, space="PSUM") as ps1,
            tc.tile_pool(name="ps2", bufs=1, space="PSUM") as ps2,
            tc.tile_pool(name="psc", bufs=1, space="PSUM") as pscp,
            tc.tile_pool(name="pseg", bufs=1, space="PSUM") as psegp,
        ):
            def load(dram, shape, dt, eng=None):
                t = cpool.tile(shape, dt, tag=f"c_{dram.name}")
                (eng or nc.sync).dma_start(t[:], dram[:])
                return t

            b1 = load(d_b1, [H, 1], f32, eng=nc.scalar)

            # region double-buffered streams
            NREG = len(regions)
            bed_t = {}
            rtt_t = {}

            def load_region(r, split=False):
                t0, nt = regions[r]
                bt = bpool.tile([128, RE_MAX], fp8, tag="bed")
                rt = rpool.tile([128, RE_MAX], fp8, tag="rtt")
                eng2 = nc.scalar if split else nc.sync
                # first region streams in group-sized slices so the first
                # matmuls start as soon as 12 tiles have landed
                step = GROUP_T if r == 0 else nt
                for o in range(0, nt, step):
                    n = min(step, nt - o)
                    esl = slice((t0 + o) * TILE_E, (t0 + o + n) * TILE_E)
                    lsl = slice(o * TILE_E, (o + n) * TILE_E)
                    nc.sync.dma_start(bt[:, lsl], d_bedT[:, esl])
                    eng2.dma_start(rt[:, lsl], d_rtT[:, esl])
                bed_t[r] = bt
                rtt_t[r] = rt

            load_region(0, split=True)
            A_sb = cpool.tile([128, NW * H], fp8, tag="c_A_sb")
            asb_cut = min(8, NW) * H
            nc.sync.dma_start(A_sb[:, 0:asb_cut], d_Asb[:, 0:asb_cut])
            cblob = load(d_cblob, [128, 648 + 2 * T], mybir.dt.uint8)
            iota = cblob[:, 0:256].bitcast(bf16)
            ident = cblob[:, 256:384].bitcast(fp8)
            w2 = cblob[:, 384:640].bitcast(bf16)
            w3 = cblob[:, 640:642].bitcast(bf16)
            b2 = cblob[:, 644:648].bitcast(f32)
            lrow = cblob[:, 648:648 + 2 * T].bitcast(bf16)
            cdt = load(d_cdt, [128, T, 3], bf16)
            if NREG > 1:
                load_region(1, split=True)
            if asb_cut < NW * H:
                nc.sync.dma_start(A_sb[:, asb_cut:], d_Asb[:, asb_cut:])
            # trigger the Silu ACT table load now that all ramp DMA-gen
            # instructions are queued (table DMAs overlap data in flight)
            actwarm = cpool.tile([128, 1], bf16, tag="actwarm")
            nc.vector.memset(actwarm[:], 0.0)
            nc.scalar.activation(actwarm[:], actwarm[:], SILU)
            # keep the PE busy through the DMA ramp so HAM is warm (2.4GHz)
            # when the first real matmuls issue
            pewarm = cpool.tile([128, 128], bf16, tag="pewarm")
            nc.vector.memset(pewarm[:], 0.0)
            pswarm = ps1.tile([128, 512], f32, tag="p1")
            for _ in range(75):
                nc.tensor.matmul(pswarm[:, 0:128], pewarm[:], pewarm[:],
                                 start=True, stop=True,
                                 skip_group_check=True)

            agg = cpool.tile([128, NW * 3], f32, tag="agg")
            nc.vector.memset(agg[:], 0.0)

            pseg_live = [None]
            pending = None

            def emit_tail(g, x1, otr_g):
                t0 = g * GROUP_T
                p2 = ps2.tile([128, GR_E], f32, tag="p2")
                for c in range(GROUP_C):
                    nc.tensor.matmul(p2[:, c * CH_E:(c + 1) * CH_E], w2[:],
                                     x1[:, c * CH_E:(c + 1) * CH_E],
                                     start=True, stop=True,
                                     skip_group_check=True)
                x2 = x2pool.tile([128, GR_E], bf16, tag="x2")
                nc.scalar.activation(x2[:], p2[:], SILU, bias=b2[:])

                psc = pscp.tile([128, GROUP_T], f32, tag="psc")
                for j in range(GROUP_T):
                    if t0 + j >= n_real_t:
                        break
                    nc.tensor.matmul(
                        psc[:, j:j + 1],
                        x2[:, j * TILE_E:(j + 1) * TILE_E],
                        w3[:], start=True, stop=True,
                        skip_group_check=True)
                if "noseg" in ABL:
                    return
                nj = min(GROUP_T, n_real_t - t0)
                cds = spool.tile([128, GROUP_T, 3], fp8, tag="cds")
                nc.vector.tensor_tensor(
                    cds[:, :nj, :], cdt[:, t0:t0 + nj, :],
                    psc[:, :nj, None].broadcast_to([128, nj, 3]),
                    AOP.mult)
                for j in range(nj):
                    gt = t0 + j
                    w, first, last = tile_window(gt)
                    if first:
                        npseg = psegp.tile([128, 3], f32, tag="pseg")
                        pseg_live[0] = npseg
                    ps = pseg_live[0]
                    nc.tensor.matmul(
                        ps[:], otr_g[:, j, :], cds[:, j, :],
                        start=first, stop=last, skip_group_check=True)
                    if last:
                        sl = agg[:, w * 3:w * 3 + 3]
                        nc.vector.tensor_add(sl, sl, ps[:])

            NGRP = T // GROUP_T
            reg_of = {}
            for ri, (rt, rn) in enumerate(regions):
                for tt in range(rt, rt + rn):
                    reg_of[tt] = ri
            for g in range(NGRP):
                t0 = g * GROUP_T
                r = reg_of[t0]
                rt0 = regions[r][0]
                if t0 >= n_real_t:
                    break
                if t0 == rt0 and r + 2 <= NREG - 1:
                    load_region(r + 2)
                off = (t0 - rt0) * TILE_E

                # one-hot [e, slot] for this group's seg matmuls (DVE)
                otr_g = opool.tile([128, GROUP_T, TILE_E], fp8, tag="otr")
                nc.vector.tensor_tensor(
                    otr_g[:],
                    iota[:, None, :].broadcast_to([128, GROUP_T, TILE_E]),
                    lrow[:, t0:t0 + GROUP_T, None].broadcast_to(
                        [128, GROUP_T, TILE_E]),
                    AOP.is_equal)

                p1 = ps1.tile([128, GR_E], f32, tag="p1")
                bed = bed_t[r]
                rtt = rtt_t[r]
                # B-insert opens each bank; A-select accumulates and closes
                for c in range(GROUP_C):
                    co = c * CH_E
                    nc.tensor.matmul(
                        p1[:, co:co + CH_E], ident[:],
                        bed[:, off + co:off + co + CH_E],
                        start=True, stop=False, skip_group_check=True)
                for c in range(GROUP_C):
                    tc0 = t0 + c * CHUNK_T
                    runs = []
                    for t in range(CHUNK_T):
                        gt = tc0 + t
                        w = 0 if gt >= n_real_t else gt // TPW
                        if runs and runs[-1][0] == w:
                            runs[-1][2] = t + 1
                        else:
                            runs.append([w, t, t + 1])
                    co = c * CH_E
                    for w, ta, tb in runs:
                        nc.tensor.matmul(
                            p1[:, co + ta * TILE_E:co + tb * TILE_E],
                            A_sb[:, w * H:(w + 1) * H],
                            rtt[:, off + co + ta * TILE_E:
                                   off + co + tb * TILE_E],
                            start=False, stop=True, skip_group_check=True)

                x1 = x1pool.tile([128, GR_E], bf16, tag="x1")
                nc.scalar.activation(x1[:], p1[:], SILU, bias=b1[:])

                if pending is not None:
                    emit_tail(*pending)
                pending = (g, x1, otr_g)

            coordm = load(d_coordm, [128, NW * 3], f32)
            masks = load(d_masks, [128, NW * 3], f32)
            # first-half output overlaps the last pipeline groups (its agg
            # windows completed mid-run); second half after the final tail
            outs = cpool.tile([128, NW * 3], f32, tag="outs")
            hcut = (NW // 2) * 3
            nc.vector.tensor_mul(outs[:, :hcut], agg[:, :hcut],
                                 masks[:, :hcut])
            nc.vector.tensor_add(outs[:, :hcut], outs[:, :hcut],
                                 coordm[:, :hcut])
            nc.sync.dma_start(d_out[:, :hcut], outs[:, :hcut])
            if pending is not None:
                emit_tail(*pending)
            nc.vector.tensor_mul(outs[:, hcut:], agg[:, hcut:],
                                 masks[:, hcut:])
            nc.vector.tensor_add(outs[:, hcut:], outs[:, hcut:],
                                 coordm[:, hcut:])
            nc.sync.dma_start(d_out[:, hcut:], outs[:, hcut:])

    nc.compile()
    return nc


# ----------------------------------------------------------------------------
# Entry point
# ----------------------------------------------------------------------------

LAST_RESULTS = None


def _ensure_ntff_hook():
    """Register the axon NTFF profile hook if the image lacks antenv.axon_hooks."""
    import types
    try:
        from antenv.axon_hooks import get_axon_ntff_profile_hook  # noqa: F401
        return
    except ImportError:
        pass
    holder = {}
    mod = types.ModuleType("antenv.axon_hooks")
    mod.set_axon_ntff_profile_hook = lambda h: holder.__setitem__("h", h)
    mod.get_axon_ntff_profile_hook = lambda: holder.get("h")
    sys.modules["antenv.axon_hooks"] = mod
    try:
        sys.path.insert(0, "/root/.axon_site")
        from trn_agent_boot.trn_boot import _ntff_profile_via_ctypes
        hook = _ntff_profile_via_ctypes("/opt/axon/libaxon_pjrt.so")
        if hook is not None:
            mod.set_axon_ntff_profile_hook(hook)
    except Exception as e:  # degrade to no trace
        print("ntff hook setup failed:", e)
    from concourse import bass_utils as _bu
    _bu.upload_artifacts = lambda tmpdir: f"local:{tmpdir}"


def kernel(**inputs):
    global LAST_RESULTS
    from concourse.bass_utils import run_bass_kernel_spmd

    in_maps, metas, dims = prep_host(**inputs)
    nc = build_program(dims)
    trace = bool(os.environ.get("KERNEL_TRACE"))
    if trace:
        _ensure_ntff_hook()
    tmpdir = os.environ.get("KERNEL_TRACE_DIR") or None
    res = run_bass_kernel_spmd(nc, in_maps, list(range(NCORES)), trace=trace,
                               tmpdir=tmpdir)
    LAST_RESULTS = res

    N = dims["N"]
    NW = dims["NW"]
    out = np.zeros((N, 3), np.float32)
    for c in range(NCORES):
        o = res.results[c]["out"].reshape(128, NW, 3)
        for w, (base, span) in enumerate(metas[c]["wins"]):
            out[base:base + span] = o[:span, w, :]
    return out
